# revision 1
# baseline (speedup 1.0000x reference)
"""Multi-head attention (B=8, S=2048, D=1024, H=16, DK=64) on 8 TRN2 NeuronCores.

Sharding: pure batch data-parallel — core i computes batch i's full attention.
No collectives needed; per-core output is the final [S, D] slice.

Per-core pipeline (all matmuls bf16, fp32 PSUM accumulation):
  1. gpsimd cast-DMA inputs f32->bf16 into DRAM staging, then HW DMA-transpose
     loads to get qT/kT/vT in [D, S] SBUF layout (contraction dim on partitions).
  2. Projections with head-PAIR packed weights: lhsT = [d, 2*64] so one matmul
     yields two heads' projected rows. q/k projected transposed [dk, s]; v
     projected natural [t, dk] with a ones column appended (softmax denominator
     comes out of the attention*V matmul for free).
  3. Scores computed transposed: scoresT[t, s] = kT_h.T @ qT_h, two heads
     row-packed into array rows 0-63 / 64-127 (K=64 each, concurrent).
  4. exp((1/32)*x) fused on ScalarE reading PSUM [128, 1024], writing bf16.
  5. AV: lhsT = [v_h | ones] [t, 65] -> out rows 0-63 = out_hT, row 64 = denom.
  6. normalize: reciprocal(denom) -> gpsimd partition_broadcast -> multiply;
     final Wo projection from transposed out tiles, interleaved with the next
     s-chunk's attention to keep ScalarE fed.
"""

import sys

if "/opt/trn_rl_repo" not in sys.path:
    sys.path.insert(0, "/opt/trn_rl_repo")

import functools
from contextlib import ExitStack

import numpy as np

import concourse.bass as bass
import concourse.mybir as mybir
import concourse.tile as tile
from concourse import bacc
from concourse.bass_utils import run_bass_kernel_spmd

F32 = mybir.dt.float32
BF16 = mybir.dt.bfloat16
P = 128

B, D, H, DK = 8, 1024, 16, 64
S_FULL = 2048
NPAIR = H // 2  # 8 head pairs
DT = D // P  # 8 d-tiles (contraction tiles for projections)
N_CORES = 8


def _body(ctx: ExitStack, tc: tile.TileContext, S: int):
    nc = tc.nc
    TT = S // P  # t-tiles
    SCW = min(1024, S)  # attention s-chunk width
    SC = S // SCW  # number of s chunks
    W5 = min(512, S)  # matmul free-dim width (one PSUM bank)
    NH = SCW // W5  # W5-wide halves per chunk

    q_ap = nc.dram_tensor("q", [S, D], F32, kind="ExternalInput").ap()
    k_ap = nc.dram_tensor("k", [S, D], F32, kind="ExternalInput").ap()
    v_ap = nc.dram_tensor("v", [S, D], F32, kind="ExternalInput").ap()
    wq_ap = nc.dram_tensor("Wq", [H, D, DK], F32, kind="ExternalInput").ap()
    wk_ap = nc.dram_tensor("Wk", [H, D, DK], F32, kind="ExternalInput").ap()
    wv_ap = nc.dram_tensor("Wv", [H, D, DK], F32, kind="ExternalInput").ap()
    wo_ap = nc.dram_tensor("Wo", [D, D], F32, kind="ExternalInput").ap()
    out_ap = nc.dram_tensor("out", [S, D], F32, kind="ExternalOutput").ap()

    scale = float(D) ** -0.5

    dram = ctx.enter_context(tc.tile_pool(name="dram", bufs=1, space="DRAM"))
    consts = ctx.enter_context(tc.tile_pool(name="consts", bufs=1))
    wpool = ctx.enter_context(tc.tile_pool(name="wpool", bufs=2))
    res = ctx.enter_context(tc.tile_pool(name="res", bufs=1))
    # PSUM: "sc" = attention scores (2 x 2 banks); "av" = AV accumulators,
    # projections and the final Wo projection share it (2 x 2 banks).
    ps_sc = ctx.enter_context(tc.tile_pool(name="ps_sc", bufs=2, space="PSUM"))
    ps_av = ctx.enter_context(tc.tile_pool(name="ps_av", bufs=2, space="PSUM"))

    # ---- weights: pair-packed w[p, dt, pair, h2, dk] (bf16, cast in DMA);
    # loads are emitted just before their consumer so the gpsimd DMA queue
    # never delays the k staging chain ----
    def load_w(name, wap):
        wt = wpool.tile([P, DT, NPAIR, 2, DK], BF16, tag="w", name=name)
        srcw = wap.rearrange("h (dt p) k -> p dt h k", p=P)
        for dt_ in range(DT):
            nc.gpsimd.dma_start(wt[:, dt_], srcw[:, dt_])
        return wt

    # ---- inputs: stage bf16, transpose-load to [d, s] layout ----
    # chunked so each transpose only waits for its own staging rows
    def load_xT(xpool, x_ap, label, first_chunks=None):
        stage = dram.tile([S, D], BF16, tag=f"stage_{label}")
        n_chunks = max(1, S // 512)
        rows = S // n_chunks
        xT = xpool.tile([P, DT, S], BF16, tag="xT", name=f"{label}T")

        def emit(c):
            sl = slice(c * rows, (c + 1) * rows)
            nc.gpsimd.dma_start(stage[sl, :], x_ap[sl, :])
            # one 3D xbar transpose per chunk: xT[p, dt, s] = stage[s, dt*128+p]
            nc.sync.dma_start_transpose(xT[:, :, sl], stage[sl, :])

        if first_chunks is None:
            for c in range(n_chunks):
                emit(c)
            return xT
        first_chunks = min(first_chunks, n_chunks)
        for c in range(first_chunks):
            emit(c)

        def finish():
            for c in range(first_chunks, n_chunks):
                emit(c)

        return xT, finish

    # ---- projections (PSUM tiles on the "av" tag so the attention-score
    # pipeline's "sc" slots are free from the start) ----
    kproj = res.tile([P, NPAIR, S], BF16, tag="kproj")
    qproj = res.tile([P, NPAIR, S], BF16, tag="qproj")
    vaug = res.tile([P, H, TT, DK + 1], BF16, tag="vaug")
    nc.vector.memset(vaug[:, :, :, DK : DK + 1], 1.0)

    ones_sb = consts.tile([1, DK], BF16, tag="ones")
    nc.vector.memset(ones_sb[:], 1.0)
    # dummy exp: pulls the ~1.3us activation-table load into startup idle
    # instead of paying it before the first real exp on the critical path
    warm_sb = consts.tile([1, 32], F32, tag="warm")
    nc.vector.memset(warm_sb[:], 0.0)
    nc.scalar.activation(
        warm_sb[:], warm_sb[:], mybir.ActivationFunctionType.Exp, scale=1.0
    )

    def project_T(xT, w, dst, pools=None):
        # dst[h2*64+dk, pair, s] = sum_d w[d, pair, h2, dk] * xT[d, s]
        # scq outer: group (scq, pr) only needs transpose chunk scq, so the
        # PE consumption rate matches the staging+transpose feed rate.
        # k/v projections may also borrow the idle "sc" slots (pools list) —
        # they retire before q-proj ends, so attention scores never wait.
        if pools is None:
            pools = [(ps_av, "av")]
        gi = 0
        for scq in range(S // W5):
            for pr in range(NPAIR):
                pool_, tag_ = pools[gi % len(pools)]
                gi += 1
                ps = pool_.tile([P, W5], F32, tag=tag_, name="proj_ps")
                for dt_ in range(DT):
                    nc.tensor.matmul(
                        ps,
                        w[:, dt_, pr],
                        xT[:, dt_, scq * W5 : (scq + 1) * W5],
                        start=dt_ == 0,
                        stop=dt_ == DT - 1,
                    )
                nc.vector.tensor_copy(
                    out=dst[:, pr, scq * W5 : (scq + 1) * W5], in_=ps
                )

    with tc.tile_pool(name="xpool", bufs=2) as xpool:
        kT, finish_k = load_xT(xpool, k_ap, "k", first_chunks=1)
        wk = load_w("wk", wk_ap)
        finish_k()
        project_T(kT, wk, kproj, pools=[(ps_av, "av"), (ps_sc, "sc")])
        # v projected natural [t, h*dk]; half-major so heads 0-7 finish first
        vT, finish_v = load_xT(xpool, v_ap, "v", first_chunks=1)
        wv = load_w("wv", wv_ap)
        finish_v()
        vgi = 0
        for half in range(2):
            for tt in range(TT):
                vpool_, vtag_ = [(ps_av, "av"), (ps_sc, "sc")][vgi % 2]
                vgi += 1
                ps = vpool_.tile([P, 512], F32, tag=vtag_, name="vproj_ps")
                for dt_ in range(DT):
                    nc.tensor.matmul(
                        ps,
                        vT[:, dt_, tt * P : (tt + 1) * P],
                        wv[:, dt_, half * 4 : (half + 1) * 4],
                        start=dt_ == 0,
                        stop=dt_ == DT - 1,
                    )
                nc.vector.tensor_copy(
                    out=vaug[:, half * 8 : (half + 1) * 8, tt, 0:DK],
                    in_=ps.rearrange("p (h k) -> p h k", k=DK),
                )

        # q last: attention for pair 0 unblocks as soon as its first q
        # slices are projected, overlapping the rest of q-proj with attention
        qT, finish_q = load_xT(xpool, q_ap, "q", first_chunks=1)
        wq = load_w("wq", wq_ap)
        finish_q()
        project_T(qT, wq, qproj)

    # xpool released; woT lives in the reclaimed space (needed only once the
    # first s-chunk finishes)
    res2 = ctx.enter_context(tc.tile_pool(name="res2", bufs=1))
    woT = res2.tile([P, DT, D], BF16, tag="woT")
    wo_stage = dram.tile([D, D], BF16, tag="wo_stage")
    nc.gpsimd.dma_start(wo_stage[:], wo_ap)
    nc.sync.dma_start_transpose(woT[:], wo_stage[:])

    apool = ctx.enter_context(tc.tile_pool(name="apool", bufs=14))
    spool = ctx.enter_context(tc.tile_pool(name="spool", bufs=1))
    fpool = ctx.enter_context(tc.tile_pool(name="fpool", bufs=3))

    # ---- attention; the previous chunk's Wo projection is interleaved into
    # the pair loop so its PSUM/PE use rides along without starving ScalarE ----
    exp_f = mybir.ActivationFunctionType.Exp

    def final_proj_step(outT_prev, sc_prev, st, dcs=None):
        s0p = sc_prev * SCW
        for dc in range(D // W5) if dcs is None else dcs:
            f_ps = ps_av.tile([P, W5], F32, tag="av", name="f_ps")
            for kt in range(DT):
                nc.tensor.matmul(
                    f_ps,
                    outT_prev[:, kt, st * P : (st + 1) * P],
                    woT[:, kt, dc * W5 : (dc + 1) * W5],
                    start=kt == 0,
                    stop=kt == DT - 1,
                )
            fo = fpool.tile([P, W5], F32, tag="fo")
            nc.vector.tensor_copy(out=fo[:], in_=f_ps[:])
            nc.sync.dma_start(
                out_ap[s0p + st * P : s0p + (st + 1) * P, dc * W5 : (dc + 1) * W5],
                fo[:],
            )

    def emit_scores(sc_, pr, tt, boost=False):
        s0 = sc_ * SCW
        sc_ps = [
            ps_sc.tile([P, SCW], F32, tag="sc", name=f"sc{h2}") for h2 in range(2)
        ]
        for h2 in range(2):
            rows = slice(h2 * DK, (h2 + 1) * DK)
            lhsT = kproj[rows, pr, tt * P : (tt + 1) * P]
            for sh in range(NH):
                mm = nc.tensor.matmul(
                    sc_ps[h2][:, sh * W5 : (sh + 1) * W5],
                    lhsT,
                    qproj[rows, pr, s0 + sh * W5 : s0 + (sh + 1) * W5],
                )
                if boost:
                    # let the scheduler run the first attention unit's scores
                    # inside the tail of the q-projection instead of after it
                    mm.ins.bass_priority = -5
        return sc_ps

    outT_prev = None
    outT = None
    st_per_pair = max(1, (SCW // P) // NPAIR)  # final-proj subtiles per pair
    units = [(sc_, pr) for sc_ in range(SC) for pr in range(NPAIR)]
    for ui, (sc_, pr) in enumerate(units):
        if pr == 0:
            outT_prev = outT
            outT = wpool.tile([P, NPAIR, SCW], BF16, tag="w", name="outT")
        boost = ui == 0
        sc_ps = emit_scores(sc_, pr, 0, boost=boost)
        # weave the previous chunk's output projection: PSUM tiles allocated
        # at PAIR START (slots freed by the PREVIOUS pair's normalize), with
        # the matmuls emitted 2-at-a-time inside the tt loop so they ride the
        # PE's slack instead of ever blocking the next scores
        fps_steps = []
        if outT_prev is not None and st_per_pair == 1:
            st = pr
            s0p = (sc_ - 1) * SCW
            for dc in range(D // W5):
                f_ps = ps_av.tile([P, W5], F32, tag="av", name="f_ps")

                def mk(f_ps=f_ps, dc=dc, st=st, s0p=s0p):
                    def step(g):
                        for kt in (g, g + 1):
                            nc.tensor.matmul(
                                f_ps,
                                outT_prev[:, kt, st * P : (st + 1) * P],
                                woT[:, kt, dc * W5 : (dc + 1) * W5],
                                start=kt == 0,
                                stop=kt == DT - 1,
                            )
                        if g + 2 >= DT:
                            fo = fpool.tile([P, W5], F32, tag="fo")
                            nc.vector.tensor_copy(out=fo[:], in_=f_ps[:])
                            nc.sync.dma_start(
                                out_ap[
                                    s0p + st * P : s0p + (st + 1) * P,
                                    dc * W5 : (dc + 1) * W5,
                                ],
                                fo[:],
                            )
                    return step

                step = mk()
                for g in range(0, DT, 2):
                    fps_steps.append((step, g))
        av_ps = [
            ps_av.tile([DK + 1, SCW], F32, tag="av", name=f"av{h2}")
            for h2 in range(2)
        ]
        # software-pipelined: the next exp's scores (including the next
        # pair's first t-tile) are always emitted before AV / normalize /
        # final-proj matmuls, so ScalarE's next input is never queued
        # behind them on the PE
        for tt in range(TT):
            ats = []
            for h2 in range(2):
                at = apool.tile([P, SCW], BF16, tag="attn", name="at")
                ei = nc.scalar.activation(at[:], sc_ps[h2][:], exp_f, scale=scale)
                if boost:
                    ei.ins.bass_priority = -5
                ats.append(at)
            if tt + 1 < TT:
                sc_ps = emit_scores(sc_, pr, tt + 1, boost=boost)
            if tt < len(fps_steps):
                fps_steps[tt][0](fps_steps[tt][1])
            for h2 in range(2):
                va = vaug[:, 2 * pr + h2, tt, :]
                for sh in range(NH):
                    nc.tensor.matmul(
                        av_ps[h2][:, sh * W5 : (sh + 1) * W5],
                        va,
                        ats[h2][:, sh * W5 : (sh + 1) * W5],
                        start=tt == 0,
                        stop=tt == TT - 1,
                    )
        # normalize: out_hT = av[0:64] * (1 / av[64]) broadcast over rows
        for h2 in range(2):
            rec = spool.tile([1, SCW], F32, tag="rec")
            nc.vector.reciprocal(rec[:], av_ps[h2][DK : DK + 1, :])
            recb = spool.tile([1, SCW], BF16, tag="recb")
            nc.vector.tensor_copy(out=recb[:], in_=rec[:])
            bc_sb = spool.tile([DK, SCW], BF16, tag="bc_sb")
            nc.gpsimd.partition_broadcast(bc_sb[:], recb[:])
            nc.vector.tensor_tensor(
                outT[h2 * DK : (h2 + 1) * DK, pr, :],
                av_ps[h2][0:DK, :],
                bc_sb[:],
                mybir.AluOpType.mult,
            )
        if outT_prev is not None and st_per_pair != 1:
            for i in range(st_per_pair):
                st = pr * st_per_pair + i
                if st < SCW // P:
                    final_proj_step(outT_prev, sc_ - 1, st)

    for st in range(SCW // P):
        final_proj_step(outT, SC - 1, st)


@functools.lru_cache(maxsize=2)
def build(S: int = S_FULL):
    nc = bacc.Bacc("TRN2", target_bir_lowering=False, debug=False)
    with tile.TileContext(nc) as tc:
        with ExitStack() as ctx:
            _body(ctx, tc, S)
    nc.compile()
    return nc


def kernel(**inputs: np.ndarray) -> np.ndarray:
    query = np.ascontiguousarray(inputs["query"], dtype=np.float32)
    key = np.ascontiguousarray(inputs["key"], dtype=np.float32)
    value = np.ascontiguousarray(inputs["value"], dtype=np.float32)
    Wq = np.ascontiguousarray(inputs["Wq"], dtype=np.float32)
    Wk = np.ascontiguousarray(inputs["Wk"], dtype=np.float32)
    Wv = np.ascontiguousarray(inputs["Wv"], dtype=np.float32)
    Wo = np.ascontiguousarray(inputs["Wo"], dtype=np.float32)

    nc = build(S_FULL)
    in_maps = [
        {
            "q": query[i],
            "k": key[i],
            "v": value[i],
            "Wq": Wq,
            "Wk": Wk,
            "Wv": Wv,
            "Wo": Wo,
        }
        for i in range(N_CORES)
    ]
    res = run_bass_kernel_spmd(nc, in_maps, core_ids=list(range(N_CORES)))
    return np.stack([res.results[i]["out"] for i in range(N_CORES)], axis=0)


if __name__ == "__main__":
    rng = np.random.default_rng(0)
    ins = {
        "query": rng.standard_normal((B, S_FULL, D), dtype=np.float32),
        "key": rng.standard_normal((B, S_FULL, D), dtype=np.float32),
        "value": rng.standard_normal((B, S_FULL, D), dtype=np.float32),
        "Wq": rng.standard_normal((H, D, DK), dtype=np.float32) * 0.02,
        "Wk": rng.standard_normal((H, D, DK), dtype=np.float32) * 0.02,
        "Wv": rng.standard_normal((H, D, DK), dtype=np.float32) * 0.02,
        "Wo": rng.standard_normal((D, D), dtype=np.float32) * 0.02,
    }
    out = kernel(**ins)
    print(out.shape, out.dtype)



# revision 39
# speedup vs baseline: 1.2120x; 1.2120x over previous
"""Multi-head attention (B=8, S=2048, D=1024, H=16, DK=64) on 8 TRN2 NeuronCores.

Sharding: pure batch data-parallel - core i computes batch i's full attention.

Per-core pipeline:
  1. q/k projected in fp8e4m3 with DoubleRow matmuls (0.5 cyc/row): weights
     quantized at x32 scale, activations cast bf16->fp8; projection PSUM
     re-quantized to fp8 at 1/8 scale for the score matmuls. Error on this
     path is damped by the 1/32 softmax scale + row normalization.
  2. Scores per (head, t-tile) as fp8 DoubleRow matmuls with K=32x2 (dk
     halves side by side in the same partitions); heads partition-packed
     4-per-128, addressed via explicit tile_position.
  3. exp() split across engines with SEPARATE PSUM pools per consumer (a
     shared pool ping-pongs producer/consumer semaphores and caps the exp
     cadence): ScalarE runs exact Exp on [128,1024] tiles; DVE tiles use
     exp(y) ~= (1+y/2)^2 (tensor_scalar affine + 2x tensor_tensor square,
     bf16) on [128,512] tiles. Row normalization cancels the quadratic's
     common-mode error.
  4. AV in natural orientation (attn slice stationary, [v|1] moving, ap=65)
     - half the PE cost of the transposed form; denominator rides in col 64.
     v/AV/Wo stay bf16 (fp8 errors would pass straight through there).
  5. Per-head normalize: DVE reciprocal + GPSIMD broadcast multiply; one
     SBUF->SBUF xbar transpose per head-pair; Wo (chunk c) woven into chunk
     c+1's attention as 2 slots per head; v-projection woven into chunk 0
     with heads processed [14, 15, 0..13] so each head's vaug columns land
     before its AV needs them.
"""

import sys

if "/opt/trn_rl_repo" not in sys.path:
    sys.path.insert(0, "/opt/trn_rl_repo")

import functools
from contextlib import ExitStack

import numpy as np

import concourse.bass as bass
import concourse.mybir as mybir
import concourse.tile as tile
from concourse import bacc
from concourse.bass_utils import run_bass_kernel_spmd

F32 = mybir.dt.float32
BF16 = mybir.dt.bfloat16
F8 = mybir.dt.float8e4
DR = mybir.MatmulPerfMode.DoubleRow
P = 128

B, D, H, DK = 8, 1024, 16, 64
S_FULL = 2048
DT = D // P  # 8 d-tiles
N_CORES = 8

SCALE_W = 32.0  # fp8 weight quantization scale for Wq/Wk
SCALE_P = 1.0 / 8.0  # fp8 re-quantization scale for q/k projections
# exp argument = score_psum * LAM ; score_psum = (SCALE_W*SCALE_P)^2 * score
LAM = 1.0 / (SCALE_W * SCALE_P) ** 2 / (float(D) ** 0.5)

# t-tile indices (of 16) whose exp runs on DVE as (1+y/2)^2
DVE_TTS = frozenset((2, 5, 8, 11, 14))
# v-projection column parts (4 heads each)
VPARTS = ((0, 256), (256, 256), (512, 256), (768, 256))


def _body(ctx: ExitStack, tc: tile.TileContext, S: int):
    nc = tc.nc
    TT = S // P  # 16 t-tiles
    SCW = 1024  # attention s-chunk width
    NCH = S // SCW  # 2 chunks
    ST = SCW // P  # 8 s-tiles per chunk
    NM = 8  # q/k projection M-tiles: (head-group hg 0..3, dk-half 0..1)
    PRJ = 256  # projection s-chunk (DR moving = 2*256)

    q_ap = nc.dram_tensor("q", [S, D], F32, kind="ExternalInput").ap()
    k_ap = nc.dram_tensor("k", [S, D], F32, kind="ExternalInput").ap()
    v_ap = nc.dram_tensor("v", [S, D], F32, kind="ExternalInput").ap()
    wq_ap = nc.dram_tensor("Wq", [H, D, DK], F32, kind="ExternalInput").ap()
    wk_ap = nc.dram_tensor("Wk", [H, D, DK], F32, kind="ExternalInput").ap()
    wv_ap = nc.dram_tensor("Wv", [H, D, DK], F32, kind="ExternalInput").ap()
    wo_ap = nc.dram_tensor("Wo", [D, D], F32, kind="ExternalInput").ap()
    out_ap = nc.dram_tensor("out", [S, D], F32, kind="ExternalOutput").ap()

    exp_f = mybir.ActivationFunctionType.Exp

    dram = ctx.enter_context(tc.tile_pool(name="dram", bufs=1, space="DRAM"))
    consts = ctx.enter_context(tc.tile_pool(name="consts", bufs=1))
    qk8 = ctx.enter_context(tc.tile_pool(name="qk8", bufs=1))
    pool_c0 = ctx.enter_context(tc.tile_pool(name="pool_c0", bufs=1))

    # dummy exp pulls the activation-table load into startup idle
    warm_sb = consts.tile([1, 32], F32, tag="warm")
    nc.vector.memset(warm_sb[:], 0.0)
    nc.scalar.activation(warm_sb[:], warm_sb[:], exp_f, scale=1.0)

    # ---- staged transpose loads: x [S, D] f32 -> xT bf16 [128, dt, S].
    # Stage-cast on gpsimd SWDGE; the xbar transposes cost 14ns/32x32-tile on
    # the issuing sequencer, so they are spread across the SP + Act queues.
    def stage_chunk(xT, stage, x_ap, c, eng):
        sl = slice(c * 512, (c + 1) * 512)
        nc.gpsimd.dma_start(stage[sl, :], x_ap[sl, :])
        eng.dma_start_transpose(xT[:, :, sl], stage[sl, :])

    # ---- q/k weights: f32 DMA in natural [p, h, k] per (head-group, dt)
    # slice - head-group 3 (heads 12-15, processed first) loads before the
    # rest so the first scores are not gated on the full 23us W DMA. The
    # pair-packed permutation happens inside the ScalarE quantize op's APs:
    # W8[p, dt, m=(hg, half), (h4 k32)] = W[hg*4+h4, dt*128+p, half*32+k32]*32
    def alloc_w8(dstpool, name):
        return dstpool.tile([P, DT, NM, P], F8, tag=f"w8_{name}", name=f"w8_{name}")

    def load_w8_hg(name, wap, wstpool, w8, hg):
        for dt_ in range(DT):
            wst = wstpool.tile(
                [P, 4, DK], F32, tag="wst", name=f"wst_{name}{hg}_{dt_}", bufs=4
            )
            nc.sync.dma_start(
                wst[:],
                wap[hg * 4 : (hg + 1) * 4, dt_ * P : (dt_ + 1) * P, :].rearrange(
                    "h p k -> p h k"
                ),
            )
            in4 = wst[:].rearrange("p h4 (half k) -> p half h4 k", half=2)
            out4 = w8[:, dt_, hg * 2 : (hg + 1) * 2].rearrange(
                "p half (h4 k) -> p half h4 k", k=32
            )
            nc.gpsimd.tensor_scalar(out4, in4, SCALE_W, None, mybir.AluOpType.mult)

    # ---- fp8 DoubleRow projection of s-chunk range [sc0, sc1) ----
    # one psum tile holds 4 sequential 256-wide units (same zero region, the
    # groups run back to back); ScalarE re-quantizes the 1024-wide span.
    # GPSIMD cannot read PSUM, so this glue lives on ScalarE.
    def proj_group(pool_ps, x8, w8, p8, m, scq, tag="prj", bufs=None):
        ps = pool_ps.tile([P, 4 * PRJ], F32, tag=tag, name="prj_ps", bufs=bufs)
        for u in range(4):
            sc_ = scq * 4 + u
            for g in range(DT // 2):
                nc.tensor.matmul(
                    ps[:, u * PRJ : (u + 1) * PRJ],
                    w8[:, 2 * g : 2 * g + 2, m],
                    x8[:, 2 * g : 2 * g + 2, sc_ * PRJ : (sc_ + 1) * PRJ],
                    start=g == 0,
                    stop=g == DT // 2 - 1,
                    perf_mode=DR,
                    skip_group_check=True,
                )
        nc.scalar.mul(p8[:, m, scq * 4 * PRJ : (scq + 1) * 4 * PRJ], ps, SCALE_P)

    def project8(pool_ps, x8, w8, p8, scq0, scq1, m_order):
        for m in m_order:
            for scq in range(scq0, scq1):
                proj_group(pool_ps, x8, w8, p8, m, scq)

    qp8 = qk8.tile([P, NM, S], F8, tag="qp8")
    kp8 = qk8.tile([P, NM, S], F8, tag="kp8")
    M_ORDER = (6, 7, 4, 5, 2, 3, 0, 1)  # head-group order 3,2,1,0

    xvT = pool_c0.tile([P, DT, S], BF16, tag="xvT")
    stage_v = dram.tile([S, D], BF16, tag="stage_v")
    wv_bf = pool_c0.tile([P, DT, D], BF16, tag="wv_bf")
    with tc.tile_pool(name="xtpool", bufs=1) as xtpool, tc.tile_pool(
        name="wstpool", bufs=1
    ) as wstpool, tc.tile_pool(name="ps_prj", bufs=4, space="PSUM") as ps_prj:
        kT = xtpool.tile([P, DT, S], BF16, tag="xT", name="xT_k")
        stage_k = dram.tile([S, D], BF16, tag="stage_k")
        qT = xtpool.tile([P, DT, S], BF16, tag="xT2", name="xT_q")
        stage_q = dram.tile([S, D], BF16, tag="stage_q")
        w8k = alloc_w8(xtpool, "k")
        w8q = alloc_w8(pool_c0, "q")
        xk8 = xtpool.tile([P, DT, S], F8, tag="xk8")
        xq8 = pool_c0.tile([P, DT, S], F8, tag="x8", name="xq8")

        # gpsimd stage-cast order = DMA-device priority order: k0, q0, q1
        # feed the first scores; v/wv mid (vproj weave at ~50us); rest later
        stage_chunk(kT, stage_k, k_ap, 0, nc.sync)
        load_w8_hg("k", wk_ap, wstpool, w8k, 3)
        stage_chunk(qT, stage_q, q_ap, 0, nc.sync)
        stage_chunk(qT, stage_q, q_ap, 1, nc.sync)
        load_w8_hg("q", wq_ap, wstpool, w8q, 3)
        for dt_ in range(DT):
            nc.gpsimd.dma_start(
                wv_bf[:, dt_].rearrange("p (h k) -> p h k", k=DK),
                wv_ap[:, dt_ * P : (dt_ + 1) * P, :].rearrange("h p k -> p h k"),
            )
        stage_chunk(xvT, stage_v, v_ap, 0, nc.sync)
        stage_chunk(kT, stage_k, k_ap, 1, nc.sync)
        stage_chunk(xvT, stage_v, v_ap, 1, nc.sync)
        stage_chunk(kT, stage_k, k_ap, 2, nc.sync)
        stage_chunk(kT, stage_k, k_ap, 3, nc.sync)
        for hg in (0, 1, 2):
            load_w8_hg("k", wk_ap, wstpool, w8k, hg)
        stage_chunk(xvT, stage_v, v_ap, 2, nc.sync)
        stage_chunk(xvT, stage_v, v_ap, 3, nc.sync)
        stage_chunk(qT, stage_q, q_ap, 2, nc.sync)
        stage_chunk(qT, stage_q, q_ap, 3, nc.sync)
        for hg in (0, 1, 2):
            load_w8_hg("q", wq_ap, wstpool, w8q, hg)

        # quantize + project as transposes land, heads 12-15 (m 6,7) first
        for c in range(4):
            sl = slice(c * 512, (c + 1) * 512)
            nc.gpsimd.tensor_copy(out=xk8[:, :, sl], in_=kT[:, :, sl])
        project8(ps_prj, xk8, w8k, kp8, 0, 2, (6, 7))
        for c in range(2):
            sl = slice(c * 512, (c + 1) * 512)
            nc.gpsimd.tensor_copy(out=xq8[:, :, sl], in_=qT[:, :, sl])
        project8(ps_prj, xq8, w8q, qp8, 0, 1, (6, 7))
        project8(ps_prj, xk8, w8k, kp8, 0, 2, (4, 5, 2, 3, 0, 1))
        project8(ps_prj, xq8, w8q, qp8, 0, 1, (4, 5, 2, 3, 0, 1))
        for c in range(2, 4):
            sl = slice(c * 512, (c + 1) * 512)
            nc.gpsimd.tensor_copy(out=xq8[:, :, sl], in_=qT[:, :, sl])

    vpool = ctx.enter_context(tc.tile_pool(name="vpool", bufs=1))
    vaug = vpool.tile([P, TT, H, DK + 1], BF16, tag="vaug")
    nc.vector.memset(vaug[:, :, :, DK : DK + 1], 1.0)
    woT = vpool.tile([P, DT, D], BF16, tag="woT")
    wo_stage = dram.tile([D, D], BF16, tag="wo_stage")
    nc.gpsimd.dma_start(wo_stage[:], wo_ap)
    nc.sync.dma_start_transpose(woT[:], wo_stage[:])

    # ---- attention phase pools (created after xtpool/wstpool free) ----
    # ONE PSUM pool; per-tile bank rounding forces: scS 2x2 banks +
    # scD 2x1 + av 2 = 8 banks. vproj/fin tiles borrow the scD tag's banks.
    ps_at = ctx.enter_context(tc.tile_pool(name="ps_at", bufs=1, space="PSUM"))
    apool = ctx.enter_context(tc.tile_pool(name="apool", bufs=7))
    upool = ctx.enter_context(tc.tile_pool(name="upool", bufs=2))
    aopool = ctx.enter_context(tc.tile_pool(name="aopool", bufs=2))
    aotpool_a = ctx.enter_context(tc.tile_pool(name="aotpool_a", bufs=1))
    rpool = ctx.enter_context(tc.tile_pool(name="rpool", bufs=2))
    fpool = ctx.enter_context(tc.tile_pool(name="fpool", bufs=1))

    def emit_scores(c, h, tt):
        """Returns list of (psum_tile, col0) score pieces for (h, tt)."""
        hg, h4 = h // 4, h % 4
        rows = slice(h4 * 32, (h4 + 1) * 32)
        ms = slice(hg * 2, hg * 2 + 2)
        lhsT = kp8[rows, ms, tt * P : (tt + 1) * P]
        if tt in DVE_TTS:
            tiles = [(ps_at.tile([P, 512], F32, tag="scD", name="scD", bufs=2), half * 512) for half in range(2)]
        else:
            tiles = [(ps_at.tile([P, SCW], F32, tag="scS", name="scS", bufs=2), 0)]
        for ps, col0 in tiles:
            w = ps.shape[-1]
            for j in range(w // PRJ):
                s0 = c * SCW + col0 + j * PRJ
                nc.tensor.matmul(
                    ps[:, j * PRJ : (j + 1) * PRJ],
                    lhsT,
                    qp8[rows, ms, s0 : s0 + PRJ],
                    perf_mode=DR,
                    tile_position=(h4 * 32, 0),
                )
        return tiles

    def emit_exp(tiles, tt):
        at = apool.tile([P, SCW], BF16, tag="attn", name="at")
        if tt in DVE_TTS:
            for ps, col0 in tiles:
                u = upool.tile([P, 512], BF16, tag="u", name="u")
                nc.vector.tensor_scalar(
                    u[:], ps[:], LAM / 2.0, 1.0, mybir.AluOpType.mult, mybir.AluOpType.add
                )
                nc.vector.tensor_tensor(
                    at[:, col0 : col0 + 512], u[:], u[:], mybir.AluOpType.mult
                )
        else:
            nc.scalar.activation(at[:], tiles[0][0][:], exp_f, scale=LAM)
        return at

    # Wo projection slot i of chunk cp: st = i // 2, half = i % 2; each slot
    # covers two 256-wide oc pieces sharing one fo tile + one SP out-DMA
    def emit_fin_slot(aoT_prev, cp, i):
        st, half = i // 2, i % 2
        fo = fpool.tile([P, 512], F32, tag="fo")
        for j in range(2):
            oc = half * 2 + j
            f_ps = ps_at.tile([P, 256], F32, tag="scD", name="f_ps", bufs=2)
            for pr in range(8):
                nc.tensor.matmul(
                    f_ps,
                    aoT_prev[:, pr, st, :],
                    woT[:, pr, oc * 256 : (oc + 1) * 256],
                    start=pr == 0,
                    stop=pr == 7,
                )
            nc.scalar.copy(out=fo[:, j * 256 : (j + 1) * 256], in_=f_ps)
        nc.sync.dma_start(
            out_ap[
                cp * SCW + st * P : cp * SCW + (st + 1) * P,
                half * 512 : (half + 1) * 512,
            ],
            fo[:],
        )

    # v-projection part slots: (vtt, part) -> 8 matmuls + ScalarE copy into
    # vaug (ScalarE has slack during the weave heads; gpsimd does not)
    def emit_vproj(vtt, part):
        col0, w = VPARTS[part]
        vps = ps_at.tile([P, 256], F32, tag="scD", name="v_ps", bufs=2)
        for dt_ in range(DT):
            nc.tensor.matmul(
                vps,
                xvT[:, dt_, vtt * P : (vtt + 1) * P],
                wv_bf[:, dt_, col0 : col0 + w],
                start=dt_ == 0,
                stop=dt_ == DT - 1,
            )
        nc.vector.tensor_copy(
            out=vaug[:, vtt, col0 // DK : (col0 + w) // DK, 0:DK],
            in_=vps[:].rearrange("p (h k) -> p h k", k=DK),
        )

    # weave plan for chunk 0: head-iteration index -> list of (vtt, part).
    # part3 (heads 12-15) synced with head 14's own t-loop; then one part per
    # weave head - always complete before the first head that reads it.
    vweave = {i: [] for i in range(9)}
    for vtt in range(TT):
        vweave[0].append((vtt, 3))
    for i, vtt in enumerate(range(TT)):
        vweave[1 + i // 8].append((vtt, 2))
    for i, vtt in enumerate(range(TT)):
        vweave[3 + i // 6].append((vtt, 1))
    for i, vtt in enumerate(range(TT)):
        vweave[6 + i // 6].append((vtt, 0))

    HEADS0 = (14, 15, 12, 13, 8, 9, 10, 11, 4, 5, 6, 7, 0, 1, 2, 3)
    # late-q projection groups (m, scq=1), woven into late chunk-0 heads
    lateq = [(m, 1) for m in M_ORDER]

    aoT_prev = None
    for c in range(NCH):
        heads = HEADS0 if c == 0 else tuple(range(H))
        if c == 0:
            aotpool = aotpool_a
        aoT_c = aotpool.tile([P, 8, ST, P], BF16, tag="aoT", name="aoT")
        ao_pair = None
        for hi, h in enumerate(heads):
            if aoT_prev is not None:
                emit_fin_slot(aoT_prev, c - 1, hi)
            if hi % 2 == 0:
                ao_pair = aopool.tile([P, ST, P], BF16, tag="ao", name="ao")
            # av: 2 bank-aligned blocks of 4 st-groups (65 f32 each); PSUM
            # zero-regions are 2KB so each block's groups share one region,
            # started/stopped by the block's first/last matmul only
            av = ps_at.tile([P, 2, 512], F32, tag="av", name="av_ps")
            vslots = list(vweave.get(hi, [])) if c == 0 else []
            tiles = emit_scores(c, h, 0)
            for tt in range(TT):
                at = emit_exp(tiles, tt)
                if tt + 1 < TT:
                    tiles = emit_scores(c, h, tt + 1)
                if c == 0 and hi == 0:
                    # part3 of this vtt synced with head 14's own t-loop
                    if vslots and vslots[0][0] == tt:
                        emit_vproj(*vslots.pop(0))
                elif vslots:
                    emit_vproj(*vslots.pop(0))
                elif c == 0 and 8 <= hi < 16 and tt == 5 and lateq:
                    m_, scq_ = lateq.pop(0)
                    proj_group(ps_at, xq8, w8q, qp8, m_, scq_, tag="scS", bufs=2)
                for st in range(ST):
                    b, sl = st // 4, st % 4
                    nc.tensor.matmul(
                        av[:, b, sl * 65 : sl * 65 + 65],
                        at[:, st * P : (st + 1) * P],
                        vaug[:, tt, h, :],
                        start=tt == 0 and sl == 0,
                        stop=tt == TT - 1 and sl == 3,
                        skip_group_check=True,
                    )
            while vslots:
                emit_vproj(*vslots.pop(0))
            # normalize: ao[s, dk] = av[s, dk] / av[s, 64]
            rec = rpool.tile([P, 2, 4, 1], F32, tag="rec")
            den = av[:, :, 0:260].rearrange("p b (sl c) -> p b sl c", c=65)[:, :, :, DK]
            nc.vector.reciprocal(rec[:, :, :, 0], den)
            pcol = (hi % 2) * DK
            nc.vector.tensor_tensor(
                ao_pair[:, :, pcol : pcol + DK].rearrange(
                    "p (b sl) k -> p b sl k", b=2
                ),
                av[:, :, 0:260].rearrange("p b (sl c) -> p b sl c", c=65)[:, :, :, 0:DK],
                rec[:].broadcast_to([P, 2, 4, DK]),
                mybir.AluOpType.mult,
            )
            if hi % 2 == 1:
                # pair index by actual head ids (14,15)->7, (0,1)->0, ...
                pr = heads[hi - 1] // 2
                nc.sync.dma_start_transpose(
                    aoT_c[:, pr], ao_pair[:].rearrange("p st k -> p (st k)")
                )
        if c == 0:
            aotpool = ctx.enter_context(tc.tile_pool(name="aotpool_b", bufs=1))
        aoT_prev = aoT_c

    for i in range(2 * ST):
        emit_fin_slot(aoT_prev, NCH - 1, i)


@functools.lru_cache(maxsize=2)
def build(S: int = S_FULL):
    nc = bacc.Bacc("TRN2", target_bir_lowering=False, debug=False)
    with tile.TileContext(nc) as tc:
        with ExitStack() as ctx:
            _body(ctx, tc, S)
    nc.compile()
    return nc


def kernel(**inputs: np.ndarray) -> np.ndarray:
    query = np.ascontiguousarray(inputs["query"], dtype=np.float32)
    key = np.ascontiguousarray(inputs["key"], dtype=np.float32)
    value = np.ascontiguousarray(inputs["value"], dtype=np.float32)
    Wq = np.ascontiguousarray(inputs["Wq"], dtype=np.float32)
    Wk = np.ascontiguousarray(inputs["Wk"], dtype=np.float32)
    Wv = np.ascontiguousarray(inputs["Wv"], dtype=np.float32)
    Wo = np.ascontiguousarray(inputs["Wo"], dtype=np.float32)

    nc = build(S_FULL)
    in_maps = [
        {
            "q": query[i],
            "k": key[i],
            "v": value[i],
            "Wq": Wq,
            "Wk": Wk,
            "Wv": Wv,
            "Wo": Wo,
        }
        for i in range(N_CORES)
    ]
    res = run_bass_kernel_spmd(nc, in_maps, core_ids=list(range(N_CORES)))
    return np.stack([res.results[i]["out"] for i in range(N_CORES)], axis=0)


if __name__ == "__main__":
    rng = np.random.default_rng(0)
    ins = {
        "query": rng.standard_normal((B, S_FULL, D), dtype=np.float32),
        "key": rng.standard_normal((B, S_FULL, D), dtype=np.float32),
        "value": rng.standard_normal((B, S_FULL, D), dtype=np.float32),
        "Wq": rng.standard_normal((H, D, DK), dtype=np.float32) * 0.02,
        "Wk": rng.standard_normal((H, D, DK), dtype=np.float32) * 0.02,
        "Wv": rng.standard_normal((H, D, DK), dtype=np.float32) * 0.02,
        "Wo": rng.standard_normal((D, D), dtype=np.float32) * 0.02,
    }
    out = kernel(**ins)
    print(out.shape, out.dtype)
